# revision 4
# baseline (speedup 1.0000x reference)
"""MoE FFN (top-2 of 8 experts, 1024->4096->1024 GELU FFN) on 8 trn2 NeuronCores.

Strategy (expert parallelism, per the sharding hint):
  - Host: compute router logits (fp64) + top-2 + softmax weights; build the
    per-expert token lists (dispatch by routing index).
  - Each of the 8 cores runs ONE expert's FFN over the tokens routed to it,
    padded to a common capacity C (SPMD: same NEFF on all cores).
  - Device: yt = fc2 @ gelu(fc1 @ x + b1) + b2, computed in fp32r (TF32-like)
    matmuls with fp32 PSUM accumulation.
  - Host: weighted combine (pure gathers, no scatter).

All FLOP-heavy work (2 x 1024x4096 matmuls per token) runs on device.
"""

import os

import numpy as np

import concourse.bass as bass
import concourse.mybir as mybir
import concourse.tile as tile
from concourse import bacc
from concourse.bass_utils import run_bass_kernel_spmd

# Model dims (hardcoded per problem spec nn_MoEFFN_25744033972352)
D = 1024
H = 4096
E = 8
P = 128
CH = 384  # token chunk = matmul moving free dim (>=256 keeps fp32r at full rate)
KD = D // P  # 8  k-tiles for layer 1
KH = H // P  # 32 k-tiles for layer 2
MH = H // P  # 32 m-tiles for layer 1 output
MD = D // P  # 8  m-tiles for layer 2 output

F32 = mybir.dt.float32
F32R = mybir.dt.float32r

# exposed for test harness introspection
LAST_RESULT = None

_kernel_cache: dict[int, object] = {}


def _build_expert_kernel(C: int):
    """Per-core Bass kernel: yt[1024, C] = W2 @ gelu(W1 @ x + b1) + b2.

    Layouts (all fp32 in DRAM, bitcast to fp32r on the way into SBUF):
      xt: [P, KD, C]       xt[p, k, t]   = x[token t, k*128 + p]
      w1: [MH, P, KD, P]   w1[m, p, k, c] = W1T[k*128+p, m*128+c] = fc1_w[m*128+c, k*128+p]
      b1: [P, MH]          b1[p, m]      = fc1_b[m*128+p]
      w2: [MD, P, KH, P]   w2[m, p, k, c] = fc2_w[m*128+c, k*128+p]
      b2: [P, MD]
      yt: [MD, P, C]       yt[m, p, t]   = out[token t, m*128+p]
    """
    nc = bacc.Bacc("TRN2", target_bir_lowering=False)
    NJ = C // CH

    xt = nc.dram_tensor("xt", [P, KD, C], F32, kind="ExternalInput")
    w1 = nc.dram_tensor("w1", [MH, P, KD, P], F32, kind="ExternalInput")
    b1 = nc.dram_tensor("b1", [P, MH], F32, kind="ExternalInput")
    w2 = nc.dram_tensor("w2", [MD, P, KH, P], F32, kind="ExternalInput")
    b2 = nc.dram_tensor("b2", [P, MD], F32, kind="ExternalInput")
    yt = nc.dram_tensor("yt", [MD, P, C], F32, kind="ExternalOutput")

    with tile.TileContext(nc) as tc:
        with (
            tc.tile_pool(name="big", bufs=1) as big,
            tc.tile_pool(name="wp", bufs=6) as wp,
            tc.tile_pool(name="ytp", bufs=2) as ytp,
            tc.tile_pool(name="psum", bufs=6, space="PSUM") as psum,
        ):
            xt_sb = big.tile([P, KD, C], F32R, tag="xt")
            ht_sb = big.tile([P, KH, C], F32R, tag="ht")
            b1_sb = big.tile([P, MH], F32, tag="b1")
            b2_sb = big.tile([P, MD], F32, tag="b2")
            nc.sync.dma_start(xt_sb[:], xt[:].bitcast(F32R))
            nc.sync.dma_start(b1_sb[:], b1[:])
            nc.sync.dma_start(b2_sb[:], b2[:])

            # ---- Layer 1: ht[h, t] = gelu(sum_d W1T[d, h] * xt[d, t] + b1[h])
            for m in range(MH):
                w1_sb = wp.tile([P, KD, P], F32R, tag="w")
                nc.sync.dma_start(w1_sb[:], w1[m].bitcast(F32R))
                for j in range(NJ):
                    ps = psum.tile([P, CH], F32, tag="ps")
                    for k in range(KD):
                        nc.tensor.matmul(
                            ps[:],
                            w1_sb[:, k, :],
                            xt_sb[:, k, j * CH : (j + 1) * CH],
                            start=(k == 0),
                            stop=(k == KD - 1),
                        )
                    nc.scalar.activation(
                        ht_sb[:, m, j * CH : (j + 1) * CH],
                        ps[:],
                        mybir.ActivationFunctionType.Gelu_apprx_tanh,
                        bias=b1_sb[:, m : m + 1],
                    )

            # ---- Layer 2: yt[o, t] = sum_h W2T[h, o] * ht[h, t] + b2[o]
            # Weight k-range is streamed in 4 groups of 8 k-tiles to keep the
            # streaming pool slots uniform with layer 1.
            KG = KH // KD  # 4 groups
            for m in range(MD):
                w2_sbs = []
                for g in range(KG):
                    w2_sb = wp.tile([P, KD, P], F32R, tag="w")
                    nc.sync.dma_start(
                        w2_sb[:], w2[m][:, g * KD : (g + 1) * KD, :].bitcast(F32R)
                    )
                    w2_sbs.append(w2_sb)
                for j in range(NJ):
                    ps = psum.tile([P, CH], F32, tag="ps")
                    for k in range(KH):
                        nc.tensor.matmul(
                            ps[:],
                            w2_sbs[k // KD][:, k % KD, :],
                            ht_sb[:, k, j * CH : (j + 1) * CH],
                            start=(k == 0),
                            stop=(k == KH - 1),
                        )
                    yt_sb = ytp.tile([P, CH], F32, tag="yt")
                    nc.vector.tensor_tensor(
                        yt_sb[:],
                        ps[:],
                        b2_sb[:, m, None].to_broadcast((P, CH)),
                        mybir.AluOpType.add,
                    )
                    nc.sync.dma_start(yt[m][:, j * CH : (j + 1) * CH], yt_sb[:])

    nc.finalize()
    return nc


def _get_kernel(C: int):
    if C not in _kernel_cache:
        _kernel_cache[C] = _build_expert_kernel(C)
    return _kernel_cache[C]


def kernel(x, router_w, fc1_w, fc1_b, fc2_w, fc2_b):
    global LAST_RESULT
    x = np.asarray(x, dtype=np.float32)
    router_w = np.asarray(router_w, dtype=np.float32)
    fc1_w = np.asarray(fc1_w, dtype=np.float32)
    fc1_b = np.asarray(fc1_b, dtype=np.float32)
    fc2_w = np.asarray(fc2_w, dtype=np.float32)
    fc2_b = np.asarray(fc2_b, dtype=np.float32)

    B, S, _D = x.shape
    T = B * S
    flat = x.reshape(T, D)

    # ---- Router (host, fp64 for stable top-k; min 2nd-3rd logit gap >> fp32 eps)
    logits = flat.astype(np.float64) @ router_w.astype(np.float64).T  # [T, E]
    idx1 = np.argmax(logits, axis=1)
    l1 = logits[np.arange(T), idx1]
    masked = logits.copy()
    masked[np.arange(T), idx1] = -np.inf
    idx2 = np.argmax(masked, axis=1)
    l2 = masked[np.arange(T), idx2]
    # softmax over the two selected logits
    mx = np.maximum(l1, l2)
    e1 = np.exp(l1 - mx)
    e2 = np.exp(l2 - mx)
    z = e1 + e2
    wt1 = (e1 / z).astype(np.float32)
    wt2 = (e2 / z).astype(np.float32)

    # ---- Dispatch: group (token, slot) pairs by expert
    toks = np.concatenate([np.arange(T), np.arange(T)])
    exps = np.concatenate([idx1, idx2])
    order = np.argsort(exps, kind="stable")
    sorted_toks = toks[order]
    counts = np.bincount(exps, minlength=E)
    starts = np.zeros(E, dtype=np.int64)
    starts[1:] = np.cumsum(counts)[:-1]
    rank = np.empty(2 * T, dtype=np.int64)
    rank[order] = np.arange(2 * T)
    pos = rank - starts[exps]  # row index of each pair within its expert's batch

    C = int(max(CH, -(-counts.max() // CH) * CH))  # round capacity up to CH multiple
    nc = _get_kernel(C)

    # ---- Build per-core inputs
    in_maps = []
    for e in range(E):
        sel = sorted_toks[starts[e] : starts[e] + counts[e]]
        xe = np.zeros((C, D), dtype=np.float32)
        xe[: counts[e]] = flat[sel]
        xt_dev = np.ascontiguousarray(xe.reshape(C, KD, P).transpose(2, 1, 0))
        w1_dev = np.ascontiguousarray(
            fc1_w[e].reshape(MH, P, KD, P).transpose(0, 3, 2, 1)
        )
        w2_dev = np.ascontiguousarray(
            fc2_w[e].reshape(MD, P, KH, P).transpose(0, 3, 2, 1)
        )
        b1_dev = np.ascontiguousarray(fc1_b[e].reshape(MH, P).T)
        b2_dev = np.ascontiguousarray(fc2_b[e].reshape(MD, P).T)
        in_maps.append(
            {"xt": xt_dev, "w1": w1_dev, "b1": b1_dev, "w2": w2_dev, "b2": b2_dev}
        )

    # ---- Run on all 8 cores
    res = run_bass_kernel_spmd(nc, in_maps, core_ids=list(range(E)))
    LAST_RESULT = res

    # ---- Combine (pure gathers): out[t] = wt1[t]*Y[e1,pos1] + wt2[t]*Y[e2,pos2]
    Y = np.stack(
        [res.results[e]["yt"].reshape(D, C).T for e in range(E)]
    )  # [E, C, D] rows=tokens
    c1 = Y[idx1, pos[:T]]
    c2 = Y[idx2, pos[T:]]
    out = wt1[:, None] * c1 + wt2[:, None] * c2
    return out.reshape(B, S, D).astype(np.float32)


# revision 10
# speedup vs baseline: 1.0577x; 1.0577x over previous
"""MoE FFN (top-2 of 8 experts, 1024->4096->1024 GELU FFN) on 8 trn2 NeuronCores.

Strategy (expert parallelism, per the sharding hint):
  - Host: compute router logits (fp64) + top-2 + softmax weights; build the
    per-expert token lists (dispatch by routing index).
  - Each of the 8 cores runs ONE expert's FFN over the tokens routed to it,
    padded to a common capacity C (SPMD: same NEFF on all cores).
  - Device: yt = fc2 @ gelu(fc1 @ x + b1) + b2, computed in fp32r (TF32-like)
    matmuls with fp32 PSUM accumulation.
  - Host: weighted combine (pure gathers, no scatter).

All FLOP-heavy work (2 x 1024x4096 matmuls per token) runs on device.
"""

import os

import numpy as np

import concourse.bass as bass
import concourse.mybir as mybir
import concourse.tile as tile
from concourse import bacc
from concourse.bass_utils import run_bass_kernel_spmd

# Model dims (hardcoded per problem spec nn_MoEFFN_25744033972352)
D = 1024
H = 4096
E = 8
P = 128
KD = D // P  # 8  k-tiles for layer 1
KH = H // P  # 32 k-tiles for layer 2
MH = H // P  # 32 m-tiles for layer 1 output
MD = D // P  # 8  m-tiles for layer 2 output


def _capacity(max_count: int) -> tuple[int, int, int]:
    """Pick capacity C = NJ * CH with 256 <= CH <= 512 and C >= max_count,
    minimizing C (matmul moving free dim CH >= 256 keeps fp32r at full rate)."""
    best = None
    for nj in range(1, 17):
        ch = -(-max_count // nj)  # ceil
        if ch > 512:
            continue
        ch = max(ch, 256)
        c = nj * ch
        if best is None or c < best[0]:
            best = (c, nj, ch)
    assert best is not None
    return best

F32 = mybir.dt.float32
F32R = mybir.dt.float32r

# exposed for test harness introspection
LAST_RESULT = None

_kernel_cache: dict[int, object] = {}


def _build_expert_kernel(C: int, NJ: int, CH: int):
    """Per-core Bass kernel: yt[1024, C] = W2 @ gelu(W1 @ x + b1) + b2.

    Layouts (all fp32 in DRAM, bitcast to fp32r on the way into SBUF):
      xt: [NJ, P, KD, CH]  xt[j, p, k, t] = x[token j*CH+t, k*128 + p]
      w1: [MH, P, KD, P]   w1[m, p, k, c] = W1T[k*128+p, m*128+c] = fc1_w[m*128+c, k*128+p]
      b1: [P, MH]          b1[p, m]      = fc1_b[m*128+p]
      w2: [MD, P, KH, P]   w2[m, p, k, c] = fc2_w[m*128+c, k*128+p]
      b2: [P, MD]
      yt: [MD, P, C]       yt[m, p, t]   = out[token t, m*128+p]
    """
    nc = bacc.Bacc("TRN2", target_bir_lowering=False)

    xt = nc.dram_tensor("xt", [NJ, P, KD, CH], F32, kind="ExternalInput")
    w1 = nc.dram_tensor("w1", [MH, P, KD, P], F32, kind="ExternalInput")
    b1 = nc.dram_tensor("b1", [P, MH], F32, kind="ExternalInput")
    w2 = nc.dram_tensor("w2", [MD, P, KH, P], F32, kind="ExternalInput")
    b2 = nc.dram_tensor("b2", [P, MD], F32, kind="ExternalInput")
    yt = nc.dram_tensor("yt", [MD, P, C], F32, kind="ExternalOutput")

    with tile.TileContext(nc) as tc:
        with (
            tc.tile_pool(name="big", bufs=1) as big,
            tc.tile_pool(name="wp", bufs=6) as wp,
            tc.tile_pool(name="ytp", bufs=2) as ytp,
            tc.tile_pool(name="psum", bufs=6, space="PSUM") as psum,
        ):
            xt_sb = big.tile([P, KD, NJ, CH], F32R, tag="xt")
            ht_sb = big.tile([P, KH, C], F32R, tag="ht")
            b1_sb = big.tile([P, MH], F32, tag="b1")
            b2_sb = big.tile([P, MD], F32, tag="b2")
            # chunk 0 first: the first matmul group only waits on this DMA
            for j in range(NJ):
                nc.sync.dma_start(xt_sb[:, :, j, :], xt[j].bitcast(F32R))
            nc.sync.dma_start(b1_sb[:], b1[:])
            nc.sync.dma_start(b2_sb[:], b2[:])

            # ---- Layer 1: ht[h, t] = gelu(sum_d W1T[d, h] * xt[d, t] + b1[h])
            # j innermost: consecutive matmuls share the same stationary weights
            for m in range(MH):
                w1_sb = wp.tile([P, KD, P], F32R, tag="w")
                nc.sync.dma_start(w1_sb[:], w1[m].bitcast(F32R))
                pss = [psum.tile([P, CH], F32, tag="ps", name=f"ps_{m}_{j}") for j in range(NJ)]
                for k in range(KD):
                    for j in range(NJ):
                        nc.tensor.matmul(
                            pss[j][:],
                            w1_sb[:, k, :],
                            xt_sb[:, k, j, :],
                            start=(k == 0),
                            stop=(k == KD - 1),
                        )
                for j in range(NJ):
                    nc.scalar.activation(
                        ht_sb[:, m, j * CH : (j + 1) * CH],
                        pss[j][:],
                        mybir.ActivationFunctionType.Gelu_apprx_tanh,
                        bias=b1_sb[:, m : m + 1],
                    )

            # ---- Layer 2: yt[o, t] = sum_h W2T[h, o] * ht[h, t] + b2[o]
            # Weight k-range is streamed in 4 groups of 8 k-tiles to keep the
            # streaming pool slots uniform with layer 1.
            KG = KH // KD  # 4 groups
            for m in range(MD):
                w2_sbs = []
                for g in range(KG):
                    w2_sb = wp.tile([P, KD, P], F32R, tag="w")
                    nc.sync.dma_start(
                        w2_sb[:], w2[m][:, g * KD : (g + 1) * KD, :].bitcast(F32R)
                    )
                    w2_sbs.append(w2_sb)
                pss = [psum.tile([P, CH], F32, tag="ps", name=f"ps_{m}_{j}") for j in range(NJ)]
                for k in range(KH):
                    for j in range(NJ):
                        nc.tensor.matmul(
                            pss[j][:],
                            w2_sbs[k // KD][:, k % KD, :],
                            ht_sb[:, k, j * CH : (j + 1) * CH],
                            start=(k == 0),
                            stop=(k == KH - 1),
                        )
                for j in range(NJ):
                    yt_sb = ytp.tile([P, CH], F32, tag="yt")
                    nc.vector.tensor_tensor(
                        yt_sb[:],
                        pss[j][:],
                        b2_sb[:, m, None].to_broadcast((P, CH)),
                        mybir.AluOpType.add,
                    )
                    nc.sync.dma_start(yt[m][:, j * CH : (j + 1) * CH], yt_sb[:])

    nc.finalize()
    return nc


def _get_kernel(C: int, NJ: int, CH: int):
    key = (C, NJ, CH)
    if key not in _kernel_cache:
        _kernel_cache[key] = _build_expert_kernel(C, NJ, CH)
    return _kernel_cache[key]


def kernel(x, router_w, fc1_w, fc1_b, fc2_w, fc2_b):
    global LAST_RESULT
    x = np.asarray(x, dtype=np.float32)
    router_w = np.asarray(router_w, dtype=np.float32)
    fc1_w = np.asarray(fc1_w, dtype=np.float32)
    fc1_b = np.asarray(fc1_b, dtype=np.float32)
    fc2_w = np.asarray(fc2_w, dtype=np.float32)
    fc2_b = np.asarray(fc2_b, dtype=np.float32)

    B, S, _D = x.shape
    T = B * S
    flat = x.reshape(T, D)

    # ---- Router (host, fp64 for stable top-k; min 2nd-3rd logit gap >> fp32 eps)
    logits = flat.astype(np.float64) @ router_w.astype(np.float64).T  # [T, E]
    idx1 = np.argmax(logits, axis=1)
    l1 = logits[np.arange(T), idx1]
    masked = logits.copy()
    masked[np.arange(T), idx1] = -np.inf
    idx2 = np.argmax(masked, axis=1)
    l2 = masked[np.arange(T), idx2]
    # softmax over the two selected logits
    mx = np.maximum(l1, l2)
    e1 = np.exp(l1 - mx)
    e2 = np.exp(l2 - mx)
    z = e1 + e2
    wt1 = (e1 / z).astype(np.float32)
    wt2 = (e2 / z).astype(np.float32)

    # ---- Dispatch: group (token, slot) pairs by expert
    toks = np.concatenate([np.arange(T), np.arange(T)])
    exps = np.concatenate([idx1, idx2])
    order = np.argsort(exps, kind="stable")
    sorted_toks = toks[order]
    counts = np.bincount(exps, minlength=E)
    starts = np.zeros(E, dtype=np.int64)
    starts[1:] = np.cumsum(counts)[:-1]
    rank = np.empty(2 * T, dtype=np.int64)
    rank[order] = np.arange(2 * T)
    pos = rank - starts[exps]  # row index of each pair within its expert's batch

    C, NJ, CH = _capacity(int(counts.max()))
    nc = _get_kernel(C, NJ, CH)

    # ---- Build per-core inputs
    in_maps = []
    for e in range(E):
        sel = sorted_toks[starts[e] : starts[e] + counts[e]]
        xe = np.zeros((C, D), dtype=np.float32)
        xe[: counts[e]] = flat[sel]
        xt_dev = np.ascontiguousarray(xe.reshape(NJ, CH, KD, P).transpose(0, 3, 2, 1))
        w1_dev = np.ascontiguousarray(
            fc1_w[e].reshape(MH, P, KD, P).transpose(0, 3, 2, 1)
        )
        w2_dev = np.ascontiguousarray(
            fc2_w[e].reshape(MD, P, KH, P).transpose(0, 3, 2, 1)
        )
        b1_dev = np.ascontiguousarray(fc1_b[e].reshape(MH, P).T)
        b2_dev = np.ascontiguousarray(fc2_b[e].reshape(MD, P).T)
        in_maps.append(
            {"xt": xt_dev, "w1": w1_dev, "b1": b1_dev, "w2": w2_dev, "b2": b2_dev}
        )

    # ---- Run on all 8 cores
    res = run_bass_kernel_spmd(nc, in_maps, core_ids=list(range(E)))
    LAST_RESULT = res

    # ---- Combine (pure gathers): out[t] = wt1[t]*Y[e1,pos1] + wt2[t]*Y[e2,pos2]
    Y = np.stack(
        [res.results[e]["yt"].reshape(D, C).T for e in range(E)]
    )  # [E, C, D] rows=tokens
    c1 = Y[idx1, pos[:T]]
    c2 = Y[idx2, pos[T:]]
    out = wt1[:, None] * c1 + wt2[:, None] * c2
    return out.reshape(B, S, D).astype(np.float32)


# revision 12
# speedup vs baseline: 1.0608x; 1.0029x over previous
"""MoE FFN (top-2 of 8 experts, 1024->4096->1024 GELU FFN) on 8 trn2 NeuronCores.

Strategy (expert parallelism, per the sharding hint):
  - Host: compute router logits (fp64) + top-2 + softmax weights; build the
    per-expert token lists (dispatch by routing index).
  - Each of the 8 cores runs ONE expert's FFN over the tokens routed to it,
    padded to a common capacity C (SPMD: same NEFF on all cores).
  - Device: yt = fc2 @ gelu(fc1 @ x + b1) + b2, computed in fp32r (TF32-like)
    matmuls with fp32 PSUM accumulation.
  - Host: weighted combine (pure gathers, no scatter).

All FLOP-heavy work (2 x 1024x4096 matmuls per token) runs on device.
"""

import os

import numpy as np

import concourse.bass as bass
import concourse.mybir as mybir
import concourse.tile as tile
from concourse import bacc
from concourse.bass_utils import run_bass_kernel_spmd

# Model dims (hardcoded per problem spec nn_MoEFFN_25744033972352)
D = 1024
H = 4096
E = 8
P = 128
KD = D // P  # 8  k-tiles for layer 1
KH = H // P  # 32 k-tiles for layer 2
MH = H // P  # 32 m-tiles for layer 1 output
MD = D // P  # 8  m-tiles for layer 2 output


def _capacity(max_count: int) -> tuple[int, int, int]:
    """Pick capacity C = NJ * CH with 256 <= CH <= 512 and C >= max_count,
    minimizing C (matmul moving free dim CH >= 256 keeps fp32r at full rate)."""
    best = None
    for nj in range(1, 17):
        ch = -(-max_count // nj)  # ceil
        if ch > 512:
            continue
        ch = max(ch, 256)
        c = nj * ch
        if best is None or c < best[0]:
            best = (c, nj, ch)
    assert best is not None
    return best

F32 = mybir.dt.float32
F32R = mybir.dt.float32r

# exposed for test harness introspection
LAST_RESULT = None

_kernel_cache: dict[int, object] = {}


def _build_expert_kernel(C: int, NJ: int, CH: int):
    """Per-core Bass kernel: yt[1024, C] = W2 @ gelu(W1 @ x + b1) + b2.

    Layouts (all fp32 in DRAM, bitcast to fp32r on the way into SBUF):
      xt: [NJ, P, KD, CH]  xt[j, p, k, t] = x[token j*CH+t, k*128 + p]
      w1: [MH, P, KD, P]   w1[m, p, k, c] = W1T[k*128+p, m*128+c] = fc1_w[m*128+c, k*128+p]
      b1: [P, MH]          b1[p, m]      = fc1_b[m*128+p]
      w2: [MD, P, KH, P]   w2[m, p, k, c] = fc2_w[m*128+c, k*128+p]
      b2: [P, MD]
      yt: [MD, P, C]       yt[m, p, t]   = out[token t, m*128+p]
    """
    nc = bacc.Bacc("TRN2", target_bir_lowering=False)

    xt = nc.dram_tensor("xt", [NJ, P, KD, CH], F32, kind="ExternalInput")
    w1 = nc.dram_tensor("w1", [MH, P, KD, P], F32, kind="ExternalInput")
    b1 = nc.dram_tensor("b1", [P, MH], F32, kind="ExternalInput")
    w2 = nc.dram_tensor("w2", [MD, P, KH, P], F32, kind="ExternalInput")
    b2 = nc.dram_tensor("b2", [P, MD], F32, kind="ExternalInput")
    yt = nc.dram_tensor("yt", [MD, P, C], F32, kind="ExternalOutput")

    with tile.TileContext(nc) as tc:
        with (
            tc.tile_pool(name="big", bufs=1) as big,
            tc.tile_pool(name="wp", bufs=6) as wp,
            tc.tile_pool(name="ytp", bufs=2) as ytp,
            tc.tile_pool(name="psum", bufs=8, space="PSUM") as psum,
        ):
            xt_sb = big.tile([P, KD, NJ, CH], F32R, tag="xt")
            ht_sb = big.tile([P, KH, C], F32R, tag="ht")
            b1_sb = big.tile([P, MH], F32, tag="b1")
            b2_sb = big.tile([P, MD], F32, tag="b2")
            # DMA emission order = ring service order. The first matmul group
            # needs only chunk 0 + w1[0]; later chunks are emitted behind the
            # first weight tiles so they stream in during the j=0 sweep.
            nc.sync.dma_start(xt_sb[:, :, 0, :], xt[0].bitcast(F32R))
            nc.sync.dma_start(b1_sb[:], b1[:])
            nc.sync.dma_start(b2_sb[:], b2[:])

            # ---- Layer 1: ht[h, t] = gelu(sum_d W1T[d, h] * xt[d, t] + b1[h])
            # Pass A: chunk j=0 only (so the PE can start as soon as chunk 0
            # lands). Pass B: remaining chunks with m reversed, reusing the
            # weight tiles of the last `bufs` m-tiles still resident in SBUF.
            w1_tiles = {}

            def l1_group(m, js, ks=None):
                pss = [
                    psum.tile([P, CH], F32, tag="ps", name=f"ps1_{m}_{j}") for j in js
                ]
                for k in range(KD):
                    for ps, j in zip(pss, js):
                        nc.tensor.matmul(
                            ps[:],
                            w1_tiles[m][:, k, :],
                            xt_sb[:, k, j, :],
                            start=(k == 0),
                            stop=(k == KD - 1),
                        )
                for ps, j in zip(pss, js):
                    nc.scalar.activation(
                        ht_sb[:, m, j * CH : (j + 1) * CH],
                        ps[:],
                        mybir.ActivationFunctionType.Gelu_apprx_tanh,
                        bias=b1_sb[:, m : m + 1],
                    )

            W_REUSE = 6  # = wp bufs: tiles of the last pass-A m-tiles stay live
            for m in range(MH):
                w1_sb = wp.tile([P, KD, P], F32R, tag="w", name=f"w1_{m}")
                nc.sync.dma_start(w1_sb[:], w1[m].bitcast(F32R))
                w1_tiles[m] = w1_sb
                if m == 1:
                    for j in range(1, NJ):
                        nc.sync.dma_start(xt_sb[:, :, j, :], xt[j].bitcast(F32R))
                l1_group(m, [0])
            if NJ > 1:
                for m in range(MH - 1, -1, -1):
                    if m < MH - W_REUSE:
                        w1_sb = wp.tile([P, KD, P], F32R, tag="w", name=f"w1b_{m}")
                        nc.sync.dma_start(w1_sb[:], w1[m].bitcast(F32R))
                        w1_tiles[m] = w1_sb
                    l1_group(m, list(range(1, NJ)))

            # ---- Layer 2: yt[o, t] = sum_h W2T[h, o] * ht[h, t] + b2[o]
            # Weight k-range is streamed in 4 groups of 8 k-tiles to keep the
            # streaming pool slots uniform with layer 1.
            KG = KH // KD  # 4 groups
            for m in range(MD):
                w2_sbs = []
                for g in range(KG):
                    w2_sb = wp.tile([P, KD, P], F32R, tag="w")
                    nc.sync.dma_start(
                        w2_sb[:], w2[m][:, g * KD : (g + 1) * KD, :].bitcast(F32R)
                    )
                    w2_sbs.append(w2_sb)
                pss = [psum.tile([P, CH], F32, tag="ps", name=f"ps_{m}_{j}") for j in range(NJ)]
                for k in range(KH):
                    for j in range(NJ):
                        nc.tensor.matmul(
                            pss[j][:],
                            w2_sbs[k // KD][:, k % KD, :],
                            ht_sb[:, k, j * CH : (j + 1) * CH],
                            start=(k == 0),
                            stop=(k == KH - 1),
                        )
                for j in range(NJ):
                    yt_sb = ytp.tile([P, CH], F32, tag="yt")
                    nc.vector.tensor_tensor(
                        yt_sb[:],
                        pss[j][:],
                        b2_sb[:, m, None].to_broadcast((P, CH)),
                        mybir.AluOpType.add,
                    )
                    nc.sync.dma_start(yt[m][:, j * CH : (j + 1) * CH], yt_sb[:])

    nc.finalize()
    return nc


def _get_kernel(C: int, NJ: int, CH: int):
    key = (C, NJ, CH)
    if key not in _kernel_cache:
        _kernel_cache[key] = _build_expert_kernel(C, NJ, CH)
    return _kernel_cache[key]


def kernel(x, router_w, fc1_w, fc1_b, fc2_w, fc2_b):
    global LAST_RESULT
    x = np.asarray(x, dtype=np.float32)
    router_w = np.asarray(router_w, dtype=np.float32)
    fc1_w = np.asarray(fc1_w, dtype=np.float32)
    fc1_b = np.asarray(fc1_b, dtype=np.float32)
    fc2_w = np.asarray(fc2_w, dtype=np.float32)
    fc2_b = np.asarray(fc2_b, dtype=np.float32)

    B, S, _D = x.shape
    T = B * S
    flat = x.reshape(T, D)

    # ---- Router (host, fp64 for stable top-k; min 2nd-3rd logit gap >> fp32 eps)
    logits = flat.astype(np.float64) @ router_w.astype(np.float64).T  # [T, E]
    idx1 = np.argmax(logits, axis=1)
    l1 = logits[np.arange(T), idx1]
    masked = logits.copy()
    masked[np.arange(T), idx1] = -np.inf
    idx2 = np.argmax(masked, axis=1)
    l2 = masked[np.arange(T), idx2]
    # softmax over the two selected logits
    mx = np.maximum(l1, l2)
    e1 = np.exp(l1 - mx)
    e2 = np.exp(l2 - mx)
    z = e1 + e2
    wt1 = (e1 / z).astype(np.float32)
    wt2 = (e2 / z).astype(np.float32)

    # ---- Dispatch: group (token, slot) pairs by expert
    toks = np.concatenate([np.arange(T), np.arange(T)])
    exps = np.concatenate([idx1, idx2])
    order = np.argsort(exps, kind="stable")
    sorted_toks = toks[order]
    counts = np.bincount(exps, minlength=E)
    starts = np.zeros(E, dtype=np.int64)
    starts[1:] = np.cumsum(counts)[:-1]
    rank = np.empty(2 * T, dtype=np.int64)
    rank[order] = np.arange(2 * T)
    pos = rank - starts[exps]  # row index of each pair within its expert's batch

    C, NJ, CH = _capacity(int(counts.max()))
    nc = _get_kernel(C, NJ, CH)

    # ---- Build per-core inputs
    in_maps = []
    for e in range(E):
        sel = sorted_toks[starts[e] : starts[e] + counts[e]]
        xe = np.zeros((C, D), dtype=np.float32)
        xe[: counts[e]] = flat[sel]
        xt_dev = np.ascontiguousarray(xe.reshape(NJ, CH, KD, P).transpose(0, 3, 2, 1))
        w1_dev = np.ascontiguousarray(
            fc1_w[e].reshape(MH, P, KD, P).transpose(0, 3, 2, 1)
        )
        w2_dev = np.ascontiguousarray(
            fc2_w[e].reshape(MD, P, KH, P).transpose(0, 3, 2, 1)
        )
        b1_dev = np.ascontiguousarray(fc1_b[e].reshape(MH, P).T)
        b2_dev = np.ascontiguousarray(fc2_b[e].reshape(MD, P).T)
        in_maps.append(
            {"xt": xt_dev, "w1": w1_dev, "b1": b1_dev, "w2": w2_dev, "b2": b2_dev}
        )

    # ---- Run on all 8 cores
    res = run_bass_kernel_spmd(nc, in_maps, core_ids=list(range(E)))
    LAST_RESULT = res

    # ---- Combine (pure gathers): out[t] = wt1[t]*Y[e1,pos1] + wt2[t]*Y[e2,pos2]
    Y = np.stack(
        [res.results[e]["yt"].reshape(D, C).T for e in range(E)]
    )  # [E, C, D] rows=tokens
    c1 = Y[idx1, pos[:T]]
    c2 = Y[idx2, pos[T:]]
    out = wt1[:, None] * c1 + wt2[:, None] * c2
    return out.reshape(B, S, D).astype(np.float32)


# revision 15
# speedup vs baseline: 1.0675x; 1.0063x over previous
"""MoE FFN (top-2 of 8 experts, 1024->4096->1024 GELU FFN) on 8 trn2 NeuronCores.

Strategy (expert parallelism, per the sharding hint):
  - Host: compute router logits (fp64) + top-2 + softmax weights; build the
    per-expert token lists (dispatch by routing index).
  - Each of the 8 cores runs ONE expert's FFN over the tokens routed to it,
    padded to a common capacity C (SPMD: same NEFF on all cores).
  - Device: yt = fc2 @ gelu(fc1 @ x + b1) + b2, computed in fp32r (TF32-like)
    matmuls with fp32 PSUM accumulation.
  - Host: weighted combine (pure gathers, no scatter).

All FLOP-heavy work (2 x 1024x4096 matmuls per token) runs on device.
"""

import os

import numpy as np

import concourse.bass as bass
import concourse.mybir as mybir
import concourse.tile as tile
from concourse import bacc
from concourse.bass_utils import run_bass_kernel_spmd

# Model dims (hardcoded per problem spec nn_MoEFFN_25744033972352)
D = 1024
H = 4096
E = 8
P = 128
KD = D // P  # 8  k-tiles for layer 1
KH = H // P  # 32 k-tiles for layer 2
MH = H // P  # 32 m-tiles for layer 1 output
MD = D // P  # 8  m-tiles for layer 2 output


def _capacity(max_count: int) -> tuple[int, int, int]:
    """Pick capacity C = NJ * CH with 256 <= CH <= 512 and C >= max_count,
    minimizing C (matmul moving free dim CH >= 256 keeps fp32r at full rate)."""
    best = None
    for nj in range(1, 17):
        ch = -(-max_count // nj)  # ceil
        if ch > 512:
            continue
        ch = max(ch, 256)
        c = nj * ch
        if best is None or c < best[0]:
            best = (c, nj, ch)
    assert best is not None
    return best

F32 = mybir.dt.float32
F32R = mybir.dt.float32r

# exposed for test harness introspection
LAST_RESULT = None

_kernel_cache: dict[int, object] = {}


def _build_expert_kernel(C: int, NJ: int, CH: int):
    """Per-core Bass kernel: yt[1024, C] = W2 @ gelu(W1 @ x + b1) + b2.

    Layouts (all fp32 in DRAM, bitcast to fp32r on the way into SBUF):
      xt: [NJ, P, KD, CH]  xt[j, p, k, t] = x[token j*CH+t, k*128 + p]
      w1: [MH, P, KD, P]   w1[m, p, k, c] = W1T[k*128+p, m*128+c] = fc1_w[m*128+c, k*128+p]
      b1: [P, MH]          b1[p, m]      = fc1_b[m*128+p]
      w2: [MD, P, KH, P]   w2[m, p, k, c] = fc2_w[m*128+c, k*128+p]
      b2: [P, MD]
      yt: [MD, P, C]       yt[m, p, t]   = out[token t, m*128+p]
    """
    nc = bacc.Bacc("TRN2", target_bir_lowering=False)

    xt = nc.dram_tensor("xt", [NJ, P, KD, CH], F32, kind="ExternalInput")
    w1 = nc.dram_tensor("w1", [MH, P, KD, P], F32, kind="ExternalInput")
    b1 = nc.dram_tensor("b1", [P, MH], F32, kind="ExternalInput")
    w2 = nc.dram_tensor("w2", [MD, P, KH, P], F32, kind="ExternalInput")
    b2 = nc.dram_tensor("b2", [P, MD], F32, kind="ExternalInput")
    yt = nc.dram_tensor("yt", [MD, P, C], F32, kind="ExternalOutput")

    with tile.TileContext(nc) as tc:
        with (
            tc.tile_pool(name="big", bufs=1) as big,
            tc.tile_pool(name="wp", bufs=8) as wp,
            tc.tile_pool(name="ytp", bufs=2) as ytp,
            tc.tile_pool(name="psum", bufs=8, space="PSUM") as psum,
        ):
            xt_sb = big.tile([P, KD, NJ, CH], F32R, tag="xt")
            ht_sb = big.tile([P, KH, C], F32R, tag="ht")
            b1_sb = big.tile([P, MH], F32, tag="b1")
            b2_sb = big.tile([P, MD], F32, tag="b2")
            # DMA emission order = ring service order. The first matmul group
            # needs only chunk 0 + w1[0]; later chunks are emitted behind the
            # first weight tiles so they stream in during the j=0 sweep.
            nc.sync.dma_start(xt_sb[:, :, 0, :], xt[0].bitcast(F32R))
            nc.sync.dma_start(b1_sb[:], b1[:])
            nc.sync.dma_start(b2_sb[:], b2[:])

            # ---- Layer 1: ht[h, t] = gelu(sum_d W1T[d, h] * xt[d, t] + b1[h])
            # Pass A: chunk j=0 only (so the PE can start as soon as chunk 0
            # lands). Pass B: remaining chunks with m reversed, reusing the
            # weight tiles of the last `bufs` m-tiles still resident in SBUF.
            w1_tiles = {}

            def l1_group(m, js, ks=None):
                pss = [
                    psum.tile([P, CH], F32, tag="ps", name=f"ps1_{m}_{j}") for j in js
                ]
                for k in range(KD):
                    for ps, j in zip(pss, js):
                        nc.tensor.matmul(
                            ps[:],
                            w1_tiles[m][:, k, :],
                            xt_sb[:, k, j, :],
                            start=(k == 0),
                            stop=(k == KD - 1),
                        )
                for ps, j in zip(pss, js):
                    nc.scalar.activation(
                        ht_sb[:, m, j * CH : (j + 1) * CH],
                        ps[:],
                        mybir.ActivationFunctionType.Gelu_apprx_tanh,
                        bias=b1_sb[:, m : m + 1],
                    )

            W_REUSE = 8  # = wp bufs: tiles of the last pass-A m-tiles stay live
            # Interleave the later chunks' xt DMAs into the weight stream in
            # quarter-chunk pieces so the weight prefetch pipeline never
            # starves for more than ~1us at a time.
            inject = []
            for j in range(1, NJ):
                for q in range(0, KD, 2):
                    inject.append((j, q))
            for m in range(MH):
                w1_sb = wp.tile([P, KD, P], F32R, tag="w", name=f"w1_{m}")
                nc.sync.dma_start(w1_sb[:], w1[m].bitcast(F32R))
                w1_tiles[m] = w1_sb
                if m >= 2 and inject:
                    j, q = inject.pop(0)
                    nc.sync.dma_start(
                        xt_sb[:, q : q + 2, j, :], xt[j][:, q : q + 2, :].bitcast(F32R)
                    )
                l1_group(m, [0])
            if NJ > 1:
                for m in range(MH - 1, -1, -1):
                    if m < MH - W_REUSE:
                        w1_sb = wp.tile([P, KD, P], F32R, tag="w", name=f"w1b_{m}")
                        nc.sync.dma_start(w1_sb[:], w1[m].bitcast(F32R))
                        w1_tiles[m] = w1_sb
                    l1_group(m, list(range(1, NJ)))

            # ---- Layer 2: yt[o, t] = sum_h W2T[h, o] * ht[h, t] + b2[o]
            # Weight k-range is streamed in 4 groups of 8 k-tiles to keep the
            # streaming pool slots uniform with layer 1.
            KG = KH // KD  # 4 groups
            for m in range(MD):
                w2_sbs = []
                for g in range(KG):
                    w2_sb = wp.tile([P, KD, P], F32R, tag="w")
                    nc.sync.dma_start(
                        w2_sb[:], w2[m][:, g * KD : (g + 1) * KD, :].bitcast(F32R)
                    )
                    w2_sbs.append(w2_sb)
                pss = [psum.tile([P, CH], F32, tag="ps", name=f"ps_{m}_{j}") for j in range(NJ)]
                for k in range(KH):
                    for j in range(NJ):
                        nc.tensor.matmul(
                            pss[j][:],
                            w2_sbs[k // KD][:, k % KD, :],
                            ht_sb[:, k, j * CH : (j + 1) * CH],
                            start=(k == 0),
                            stop=(k == KH - 1),
                        )
                for j in range(NJ):
                    yt_sb = ytp.tile([P, CH], F32, tag="yt")
                    nc.vector.tensor_tensor(
                        yt_sb[:],
                        pss[j][:],
                        b2_sb[:, m, None].to_broadcast((P, CH)),
                        mybir.AluOpType.add,
                    )
                    nc.sync.dma_start(yt[m][:, j * CH : (j + 1) * CH], yt_sb[:])

    nc.finalize()
    return nc


def _get_kernel(C: int, NJ: int, CH: int):
    key = (C, NJ, CH)
    if key not in _kernel_cache:
        _kernel_cache[key] = _build_expert_kernel(C, NJ, CH)
    return _kernel_cache[key]


def kernel(x, router_w, fc1_w, fc1_b, fc2_w, fc2_b):
    global LAST_RESULT
    x = np.asarray(x, dtype=np.float32)
    router_w = np.asarray(router_w, dtype=np.float32)
    fc1_w = np.asarray(fc1_w, dtype=np.float32)
    fc1_b = np.asarray(fc1_b, dtype=np.float32)
    fc2_w = np.asarray(fc2_w, dtype=np.float32)
    fc2_b = np.asarray(fc2_b, dtype=np.float32)

    B, S, _D = x.shape
    T = B * S
    flat = x.reshape(T, D)

    # ---- Router (host, fp64 for stable top-k; min 2nd-3rd logit gap >> fp32 eps)
    logits = flat.astype(np.float64) @ router_w.astype(np.float64).T  # [T, E]
    idx1 = np.argmax(logits, axis=1)
    l1 = logits[np.arange(T), idx1]
    masked = logits.copy()
    masked[np.arange(T), idx1] = -np.inf
    idx2 = np.argmax(masked, axis=1)
    l2 = masked[np.arange(T), idx2]
    # softmax over the two selected logits
    mx = np.maximum(l1, l2)
    e1 = np.exp(l1 - mx)
    e2 = np.exp(l2 - mx)
    z = e1 + e2
    wt1 = (e1 / z).astype(np.float32)
    wt2 = (e2 / z).astype(np.float32)

    # ---- Dispatch: group (token, slot) pairs by expert
    toks = np.concatenate([np.arange(T), np.arange(T)])
    exps = np.concatenate([idx1, idx2])
    order = np.argsort(exps, kind="stable")
    sorted_toks = toks[order]
    counts = np.bincount(exps, minlength=E)
    starts = np.zeros(E, dtype=np.int64)
    starts[1:] = np.cumsum(counts)[:-1]
    rank = np.empty(2 * T, dtype=np.int64)
    rank[order] = np.arange(2 * T)
    pos = rank - starts[exps]  # row index of each pair within its expert's batch

    # SBUF fits C up to ~1088 tokens per expert per launch; larger (unusually
    # imbalanced) routings are handled by splitting into multiple launches.
    CMAX = 1088
    max_count = int(counts.max())
    rounds = max(1, -(-max_count // CMAX))
    per_round = -(-max_count // rounds)

    w_maps = []
    for e in range(E):
        w1_dev = np.ascontiguousarray(
            fc1_w[e].reshape(MH, P, KD, P).transpose(0, 3, 2, 1)
        )
        w2_dev = np.ascontiguousarray(
            fc2_w[e].reshape(MD, P, KH, P).transpose(0, 3, 2, 1)
        )
        b1_dev = np.ascontiguousarray(fc1_b[e].reshape(MH, P).T)
        b2_dev = np.ascontiguousarray(fc2_b[e].reshape(MD, P).T)
        w_maps.append({"w1": w1_dev, "b1": b1_dev, "w2": w2_dev, "b2": b2_dev})

    Yv = [[] for _ in range(E)]  # per-expert valid output rows, in order
    for r in range(rounds):
        r_counts = [
            max(0, min(per_round, int(counts[e]) - r * per_round)) for e in range(E)
        ]
        C, NJ, CH = _capacity(max(r_counts))
        nc = _get_kernel(C, NJ, CH)
        in_maps = []
        for e in range(E):
            sel = sorted_toks[
                starts[e] + r * per_round : starts[e] + r * per_round + r_counts[e]
            ]
            xe = np.zeros((C, D), dtype=np.float32)
            xe[: r_counts[e]] = flat[sel]
            xt_dev = np.ascontiguousarray(
                xe.reshape(NJ, CH, KD, P).transpose(0, 3, 2, 1)
            )
            in_maps.append({"xt": xt_dev, **w_maps[e]})

        res = run_bass_kernel_spmd(nc, in_maps, core_ids=list(range(E)))
        LAST_RESULT = res
        for e in range(E):
            Yv[e].append(res.results[e]["yt"].reshape(D, C).T[: r_counts[e]])

    # ---- Combine (pure gathers): out[t] = wt1[t]*Y[e1,pos1] + wt2[t]*Y[e2,pos2]
    contrib = np.empty((2 * T, D), dtype=np.float32)
    for e in range(E):
        rows = np.concatenate(Yv[e], axis=0) if len(Yv[e]) > 1 else Yv[e][0]
        mask = exps == e
        if mask.any():
            contrib[mask] = rows[pos[mask]]
    out = wt1[:, None] * contrib[:T] + wt2[:, None] * contrib[T:]
    return out.reshape(B, S, D).astype(np.float32)


# revision 18
# speedup vs baseline: 1.0741x; 1.0062x over previous
"""MoE FFN (top-2 of 8 experts, 1024->4096->1024 GELU FFN) on 8 trn2 NeuronCores.

Strategy (expert parallelism, per the sharding hint):
  - Host: compute router logits (fp64) + top-2 + softmax weights; build the
    per-expert token lists (dispatch by routing index).
  - Each of the 8 cores runs ONE expert's FFN over the tokens routed to it,
    padded to a common capacity C (SPMD: same NEFF on all cores).
  - Device: yt = fc2 @ gelu(fc1 @ x + b1) + b2, computed in fp32r (TF32-like)
    matmuls with fp32 PSUM accumulation.
  - Host: weighted combine (pure gathers, no scatter).

All FLOP-heavy work (2 x 1024x4096 matmuls per token) runs on device.
"""

import os

import numpy as np

import concourse.bass as bass
import concourse.mybir as mybir
import concourse.tile as tile
from concourse import bacc
from concourse.bass_utils import run_bass_kernel_spmd

# Model dims (hardcoded per problem spec nn_MoEFFN_25744033972352)
D = 1024
H = 4096
E = 8
P = 128
KD = D // P  # 8  k-tiles for layer 1
KH = H // P  # 32 k-tiles for layer 2
MH = H // P  # 32 m-tiles for layer 1 output
MD = D // P  # 8  m-tiles for layer 2 output


def _chunks(max_count: int) -> tuple[int, ...]:
    """Split capacity C >= max_count into chunks, each in [256, 512] (matmul
    moving free dim >= 256 keeps fp32r at full rate, <= 512 fits a PSUM bank).
    The first two chunks are small (startup-critical DMA); the last is big."""
    C = max(256, max_count)
    if C <= 512:
        return (C,)
    nj = -(-C // 512)
    ch = -(-C // nj)
    return (ch,) * (nj - 1) + (C - ch * (nj - 1),)

F32 = mybir.dt.float32
F32R = mybir.dt.float32r

# exposed for test harness introspection
LAST_RESULT = None

_kernel_cache: dict[int, object] = {}


def _build_expert_kernel(chunks: tuple[int, ...]):
    """Per-core Bass kernel: yt[1024, C] = W2 @ gelu(W1 @ x + b1) + b2.

    Layouts (all fp32 in DRAM, bitcast to fp32r on the way into SBUF):
      xt: [P, KD, C]       xt[p, k, t]   = x[token t, k*128 + p]
      w1: [MH, P, KD, P]   w1[m, p, k, c] = W1T[k*128+p, m*128+c] = fc1_w[m*128+c, k*128+p]
      b1: [P, MH]          b1[p, m]      = fc1_b[m*128+p]
      w2: [MD, P, KH, P]   w2[m, p, k, c] = fc2_w[m*128+c, k*128+p]
      b2: [P, MD]
      yt: [MD, P, C]       yt[m, p, t]   = out[token t, m*128+p]

    Tokens are processed in `chunks` (each a matmul moving-free-dim). Layer 1
    runs two passes: pass A covers the first min(2, NJ) chunks (2 matmuls per
    streamed weight tile, so the PE outpaces the weight DMA and absorbs the
    last chunk's xt injection); pass B covers the last chunk with m reversed,
    reusing the last `wp bufs` weight tiles still resident in SBUF.
    """
    C = sum(chunks)
    NJ = len(chunks)
    offs = [sum(chunks[:j]) for j in range(NJ)]
    nc = bacc.Bacc("TRN2", target_bir_lowering=False)

    xt = nc.dram_tensor("xt", [P, KD, C], F32, kind="ExternalInput")
    w1 = nc.dram_tensor("w1", [MH, P, KD, P], F32, kind="ExternalInput")
    b1 = nc.dram_tensor("b1", [P, MH], F32, kind="ExternalInput")
    w2 = nc.dram_tensor("w2", [MD, P, KH, P], F32, kind="ExternalInput")
    b2 = nc.dram_tensor("b2", [P, MD], F32, kind="ExternalInput")
    yt = nc.dram_tensor("yt", [MD, P, C], F32, kind="ExternalOutput")

    passA = list(range(min(2, NJ)))
    passB = list(range(len(passA), NJ))

    with tile.TileContext(nc) as tc:
        with (
            tc.tile_pool(name="big", bufs=1) as big,
            tc.tile_pool(name="wp", bufs=8) as wp,
            tc.tile_pool(name="ytp", bufs=2) as ytp,
            tc.tile_pool(name="psum", bufs=8, space="PSUM") as psum,
        ):
            xt_sb = big.tile([P, KD, C], F32R, tag="xt")
            ht_sb = big.tile([P, KH, C], F32R, tag="ht")
            b1_sb = big.tile([P, MH], F32, tag="b1")
            b2_sb = big.tile([P, MD], F32, tag="b2")

            def xsl(sb, j):
                return sb[:, :, offs[j] : offs[j] + chunks[j]]

            # DMA emission order = ring service order: pass-A chunks first
            # (startup-critical), then biases; pass-B chunks are injected
            # into the weight stream below in k-slice pieces.
            for j in passA:
                nc.sync.dma_start(xsl(xt_sb, j), xsl(xt, j).bitcast(F32R))
            nc.sync.dma_start(b1_sb[:], b1[:])
            nc.sync.dma_start(b2_sb[:], b2[:])

            w1_tiles = {}

            def l1_group(m, js):
                pss = [
                    psum.tile([P, chunks[j]], F32, tag="ps", name=f"ps1_{m}_{j}")
                    for j in js
                ]
                for k in range(KD):
                    for ps, j in zip(pss, js):
                        nc.tensor.matmul(
                            ps[:],
                            w1_tiles[m][:, k, :],
                            xt_sb[:, k, offs[j] : offs[j] + chunks[j]],
                            start=(k == 0),
                            stop=(k == KD - 1),
                        )
                for ps, j in zip(pss, js):
                    nc.scalar.activation(
                        ht_sb[:, m, offs[j] : offs[j] + chunks[j]],
                        ps[:],
                        mybir.ActivationFunctionType.Gelu_apprx_tanh,
                        bias=b1_sb[:, m : m + 1],
                    )

            W_REUSE = 8  # = wp bufs
            inject = [(j, q) for j in passB for q in range(0, KD, 2)]
            for m in range(MH):
                w1_sb = wp.tile([P, KD, P], F32R, tag="w", name=f"w1_{m}")
                nc.sync.dma_start(w1_sb[:], w1[m].bitcast(F32R))
                w1_tiles[m] = w1_sb
                if m >= 2 and inject:
                    j, q = inject.pop(0)
                    nc.sync.dma_start(
                        xt_sb[:, q : q + 2, offs[j] : offs[j] + chunks[j]],
                        xt[:, q : q + 2, offs[j] : offs[j] + chunks[j]].bitcast(F32R),
                    )
                l1_group(m, passA)
            if passB:
                for m in range(MH - 1, -1, -1):
                    if m < MH - W_REUSE:
                        w1_sb = wp.tile([P, KD, P], F32R, tag="w", name=f"w1b_{m}")
                        nc.sync.dma_start(w1_sb[:], w1[m].bitcast(F32R))
                        w1_tiles[m] = w1_sb
                    l1_group(m, passB)

            # ---- Layer 2: yt[o, t] = sum_h W2T[h, o] * ht[h, t] + b2[o]
            KG = KH // KD  # 4 weight groups of 8 k-tiles (uniform pool slots)
            for m in range(MD):
                w2_sbs = []
                for g in range(KG):
                    w2_sb = wp.tile([P, KD, P], F32R, tag="w")
                    nc.sync.dma_start(
                        w2_sb[:], w2[m][:, g * KD : (g + 1) * KD, :].bitcast(F32R)
                    )
                    w2_sbs.append(w2_sb)
                pss = [
                    psum.tile([P, chunks[j]], F32, tag="ps", name=f"ps2_{m}_{j}")
                    for j in range(NJ)
                ]
                for k in range(KH):
                    for j in range(NJ):
                        nc.tensor.matmul(
                            pss[j][:],
                            w2_sbs[k // KD][:, k % KD, :],
                            ht_sb[:, k, offs[j] : offs[j] + chunks[j]],
                            start=(k == 0),
                            stop=(k == KH - 1),
                        )
                for j in range(NJ):
                    yt_sb = ytp.tile([P, chunks[j]], F32, tag="yt", name=f"yt_{m}_{j}")
                    nc.vector.tensor_tensor(
                        yt_sb[:],
                        pss[j][:],
                        b2_sb[:, m, None].to_broadcast((P, chunks[j])),
                        mybir.AluOpType.add,
                    )
                    nc.sync.dma_start(
                        yt[m][:, offs[j] : offs[j] + chunks[j]], yt_sb[:]
                    )

    nc.finalize()
    return nc


def _get_kernel(chunks: tuple[int, ...]):
    if chunks not in _kernel_cache:
        _kernel_cache[chunks] = _build_expert_kernel(chunks)
    return _kernel_cache[chunks]


def kernel(x, router_w, fc1_w, fc1_b, fc2_w, fc2_b):
    global LAST_RESULT
    x = np.asarray(x, dtype=np.float32)
    router_w = np.asarray(router_w, dtype=np.float32)
    fc1_w = np.asarray(fc1_w, dtype=np.float32)
    fc1_b = np.asarray(fc1_b, dtype=np.float32)
    fc2_w = np.asarray(fc2_w, dtype=np.float32)
    fc2_b = np.asarray(fc2_b, dtype=np.float32)

    B, S, _D = x.shape
    T = B * S
    flat = x.reshape(T, D)

    # ---- Router (host, fp64 for stable top-k; min 2nd-3rd logit gap >> fp32 eps)
    logits = flat.astype(np.float64) @ router_w.astype(np.float64).T  # [T, E]
    idx1 = np.argmax(logits, axis=1)
    l1 = logits[np.arange(T), idx1]
    masked = logits.copy()
    masked[np.arange(T), idx1] = -np.inf
    idx2 = np.argmax(masked, axis=1)
    l2 = masked[np.arange(T), idx2]
    # softmax over the two selected logits
    mx = np.maximum(l1, l2)
    e1 = np.exp(l1 - mx)
    e2 = np.exp(l2 - mx)
    z = e1 + e2
    wt1 = (e1 / z).astype(np.float32)
    wt2 = (e2 / z).astype(np.float32)

    # ---- Dispatch: group (token, slot) pairs by expert
    toks = np.concatenate([np.arange(T), np.arange(T)])
    exps = np.concatenate([idx1, idx2])
    order = np.argsort(exps, kind="stable")
    sorted_toks = toks[order]
    counts = np.bincount(exps, minlength=E)
    starts = np.zeros(E, dtype=np.int64)
    starts[1:] = np.cumsum(counts)[:-1]
    rank = np.empty(2 * T, dtype=np.int64)
    rank[order] = np.arange(2 * T)
    pos = rank - starts[exps]  # row index of each pair within its expert's batch

    # SBUF fits C up to ~1088 tokens per expert per launch; larger (unusually
    # imbalanced) routings are handled by splitting into multiple launches.
    CMAX = 1088
    max_count = int(counts.max())
    rounds = max(1, -(-max_count // CMAX))
    per_round = -(-max_count // rounds)

    w_maps = []
    for e in range(E):
        w1_dev = np.ascontiguousarray(
            fc1_w[e].reshape(MH, P, KD, P).transpose(0, 3, 2, 1)
        )
        w2_dev = np.ascontiguousarray(
            fc2_w[e].reshape(MD, P, KH, P).transpose(0, 3, 2, 1)
        )
        b1_dev = np.ascontiguousarray(fc1_b[e].reshape(MH, P).T)
        b2_dev = np.ascontiguousarray(fc2_b[e].reshape(MD, P).T)
        w_maps.append({"w1": w1_dev, "b1": b1_dev, "w2": w2_dev, "b2": b2_dev})

    Yv = [[] for _ in range(E)]  # per-expert valid output rows, in order
    for r in range(rounds):
        r_counts = [
            max(0, min(per_round, int(counts[e]) - r * per_round)) for e in range(E)
        ]
        chunks = _chunks(max(r_counts))
        C = sum(chunks)
        nc = _get_kernel(chunks)
        in_maps = []
        for e in range(E):
            sel = sorted_toks[
                starts[e] + r * per_round : starts[e] + r * per_round + r_counts[e]
            ]
            xe = np.zeros((C, D), dtype=np.float32)
            xe[: r_counts[e]] = flat[sel]
            xt_dev = np.ascontiguousarray(xe.reshape(C, KD, P).transpose(2, 1, 0))
            in_maps.append({"xt": xt_dev, **w_maps[e]})

        res = run_bass_kernel_spmd(nc, in_maps, core_ids=list(range(E)))
        LAST_RESULT = res
        for e in range(E):
            Yv[e].append(res.results[e]["yt"].reshape(D, C).T[: r_counts[e]])

    # ---- Combine (pure gathers): out[t] = wt1[t]*Y[e1,pos1] + wt2[t]*Y[e2,pos2]
    contrib = np.empty((2 * T, D), dtype=np.float32)
    for e in range(E):
        rows = np.concatenate(Yv[e], axis=0) if len(Yv[e]) > 1 else Yv[e][0]
        mask = exps == e
        if mask.any():
            contrib[mask] = rows[pos[mask]]
    out = wt1[:, None] * contrib[:T] + wt2[:, None] * contrib[T:]
    return out.reshape(B, S, D).astype(np.float32)
